# revision 53
# baseline (speedup 1.0000x reference)
"""AliasFreeConv Trainium2 kernel.

Data-parallel over batch: 8 samples -> 8 NeuronCores, no collectives.
Per core:
  style modulation (PE matvec) scales x per-ci; demod (from a
  host-precomputed Q[ci,co] = sum_k conv_w^2) and act bias are applied
  per-co on the conv output epilogue ->
  per-sample 3x3 VALID conv via Winograd F(2x2,3x3): input transform
  B^T d B on DVE/Pool (two 1D passes), 16 per-(xi,nu) GEMMs with co on
  PSUM partitions (bf16, ~2.25x fewer PE cycles than direct), inverse
  transform stage-1 folded into the PSUM drains, stage-2 on DVE, then
  PE identity-matmul transposes scatter Y back into [2 w-cols x 64 h,
  co] conv-output tiles ->
  separable FIR up/down resampling as dense-matrix matmuls (U: 62->128,
  D: 128->64) with DRAM corner-turns between the up stages and between
  the down stages; Prelu(sqrt2*x, 0.2) on ACT.

All heavy matmuls in bf16 (fp32 PSUM accumulation); intermediates held in
bf16 to halve HBM traffic. The paired FIR matmuls use PE row/col tiling
(lo half on partitions 0-63, hi on 64-127) so each pair runs concurrently
on the 128x128 array.
"""
import math
import os
import numpy as np
from contextlib import ExitStack

import ml_dtypes

import concourse.bass as bass
import concourse.bacc as bacc
import concourse.tile as tile
from concourse import mybir
from concourse.bass_utils import run_bass_kernel_spmd

F32 = mybir.dt.float32
BF16 = mybir.dt.bfloat16
AF = mybir.ActivationFunctionType
ALU = mybir.AluOpType

B, CI, CO, H, W = 8, 512, 512, 64, 64
KS, TAPS, UP = 3, 12, 2
HO = H - 2                      # 62 valid conv outputs per axis
STYLE = 512
XPAD = 64 * 64                  # x tile free size: row-major (h, w)
NB = HO // 2                    # 31 w-column-pair blocks (= 31 tx tiles)
K9 = KS * KS
NT = 31                         # winograd tiles per axis
NT2 = NT * NT                   # 961 tiles
LIN_SCALE = 1.0 / math.sqrt(STYLE)
WSCALE = 1.0 / math.sqrt(CI * KS * KS)
EPS = 1e-8
SQRT2 = math.sqrt(2.0)

_CACHE: dict = {}
USE_GP_OPS = True    # use GPSIMD for elementwise transform ops
USE_GP_DMA = True    # use GPSIMD (SWDGE) for DMA issues


def _build_nc(reps=1, variant=None):
    # variant: None = full kernel; "p1" = phase 1 only; "noc" = single conv
    # matmul per block; "p2" = phase 2 only; "p2g*" = phase-2 g-loop only
    # (p2gd: prelu on DVE, p2gp: no prelu, p2gs: no slab copies, p2gn: no
    # corner DMA) -- dev-only timing bisection knobs
    nc = bacc.Bacc()

    # consolidated input blobs: DMA issues cost ~1.6us fixed each on HW,
    # so small tensors ride in two blobs (f32 prologue / bf16 consts)
    xt_d = nc.declare_dram_parameter("xt", [128, 4 * XPAD], BF16, isOutput=False)
    # preb: modw[0:2048] qmat[2048:4096] stylec[4096:4100] modb[4100:4104]
    #       actb-cols[4104:4108]
    preb_d = nc.declare_dram_parameter("preb", [128, 4108], F32, isOutput=False)
    # cstb: ulo[0:128] uhi[128:256] dmat[256:320] eye[320:448]
    cstb_d = nc.declare_dram_parameter("cstb", [128, 448], BF16, isOutput=False)
    wt_d = nc.declare_dram_parameter("wt", [4, 128, 4 * 4 * CO], BF16, isOutput=False)
    out_d = nc.declare_dram_parameter("o", [64, 64, CO], BF16, isOutput=True)

    u1_d = nc.dram_tensor("u1scratch", [HO, 128, CO], BF16)
    # corner-turned activations, split by hu half so the H-down lo-half
    # loads can start while the hi-half slabs are still being produced.
    d1_d = [nc.dram_tensor(f"d1scratch{h}", [64, 64, CO], BF16)  # [wd, hu, co]
            for h in range(2)]

    with ExitStack() as ctx:
        tc = ctx.enter_context(tile.TileContext(nc))
        if reps > 1:
            ctx.enter_context(tc.For_i(0, reps, 1))
        pp = ctx.enter_context(tc.tile_pool(name="persist", bufs=1))

        cst_t = pp.tile([128, 448], BF16)
        s_sc = pp.tile([128, 4], F32)
        dcol = pp.tile([128, 4], F32)   # demod, transposed per co-chunk
        bcol = pp.tile([128, 4], F32)   # act bias, transposed per co-chunk
        nc.sync.dma_start(out=cst_t, in_=cstb_d[:, :])
        ulo_t = cst_t[:, 0:128]
        uhi_t = cst_t[:, 128:256]
        dmat_t = cst_t[:, 256:320]
        eye_t = cst_t[:, 320:448]

        # Winograd F(2x2,3x3) combos. BT rows have exactly 2 nonzeros of
        # +-1, so each 1D transform plane is a single tensor_tensor op.
        #   BT = [[1,0,-1,0],[0,1,1,0],[0,-1,1,0],[0,1,0,-1]]
        BT_COMBO = [(0, 2, ALU.subtract), (1, 2, ALU.add),
                    (2, 1, ALU.subtract), (1, 3, ALU.subtract)]

        with tc.tile_pool(name="xw", bufs=1) as xw, \
             tc.tile_pool(name="cpool", bufs=3) as cpool, \
             tc.tile_pool(name="upool", bufs=3) as upool, \
             tc.tile_pool(name="ps2", bufs=2, space="PSUM") as ps2, \
             ExitStack() as xctx, ExitStack() as yctx:
            xpool = xctx.enter_context(tc.tile_pool(name="xpool", bufs=1))
            # ---- phase 1 resident tensors ----
            xall = xpool.tile([128, 4 * XPAD], BF16, tag="x", name="x")
            xts = [xall[:, t * XPAD:(t + 1) * XPAD] for t in range(4)]
            # P accumulators (stage-1 inverse transform), per co-chunk:
            # 8 planes (r, nu) of 961 tiles each
            pts = [xw.tile([128, 8 * NT2], BF16, tag=f"p{c}", name=f"p{c}") for c in range(4)]
            # Y planes (stage-2 output): allocated in phase 1b, after the
            # 1a transform pools free their space
            yts = []

            # ---- prologue: style modulation (scales x); demod per-co from
            # host-precomputed Q[ci,co] = sum_k conv_w^2 (f32 matvecs). ----
            with tc.tile_pool(name="pre", bufs=1) as pre:
                preb_t = pre.tile([128, 4108], F32)
                nc.sync.dma_start(out=preb_t, in_=preb_d[:, :])
                modw_t = [preb_t[:, 512 * t:512 * t + 512] for t in range(4)]
                styl_t = [preb_t[:, 4096 + t:4097 + t] for t in range(4)]
                modb_t = preb_t[:, 4100:4104]
                qm_t = [preb_t[:, 2048 + 512 * t:2048 + 512 * t + 512]
                        for t in range(4)]
                ones_row = pre.tile([1, 128], F32)
                sd_row = pre.tile([1, CO], F32)
                demod_row = pre.tile([1, CO], F32)
                s2_t = pre.tile([128, 4], F32)
                eps_t = pre.tile([1, 1], F32)
                nc.vector.memset(ones_row, 1.0)
                nc.vector.memset(eps_t, EPS * (1.0 + EPS) ** 2)
                nc.vector.tensor_copy(out=bcol, in_=preb_t[:, 4104:4108])
                nc.scalar.dma_start(out=xall, in_=xt_d[:, :])

                # s = style @ (mod_w*lin_scale).T; s_sc = wscale*(s + mod_b)
                pt_s = ps2.tile([128, CO], F32, tag="uh")
                for cib in range(4):
                    for dt_ in range(4):
                        nc.tensor.matmul(pt_s[:, cib:cib + 1],
                                         modw_t[dt_][:, cib * 128:(cib + 1) * 128],
                                         styl_t[dt_], start=(dt_ == 0), stop=(dt_ == 3))
                nc.vector.scalar_tensor_tensor(
                    out=s_sc, in0=pt_s[:, 0:4], scalar=WSCALE,
                    in1=modb_t, op0=ALU.mult, op1=ALU.add)

                # x <- x * s_sc (per-ci modulation on the input, not the weights)
                for t in range(4):
                    nc.vector.tensor_scalar_mul(xts[t], xts[t], s_sc[:, t:t + 1])

                # A[co] = sum_ci s_sc[ci]^2 * Q[ci, co]; demod = rsqrt(A+eps)
                nc.scalar.activation(out=s2_t, in_=s_sc, func=AF.Square)
                pa = ps2.tile([128, CO], F32, tag="uh")
                for t in range(4):
                    nc.tensor.matmul(pa[0:1, :], s2_t[:, t:t + 1], qm_t[t],
                                     start=(t == 0), stop=(t == 3))
                c2 = (1.0 + EPS) ** 2
                nc.scalar.activation(out=sd_row, in_=pa[0:1, :], func=AF.Sqrt,
                                     scale=c2, bias=eps_t)
                nc.vector.reciprocal(demod_row, sd_row)

                # transpose the demod row into per-co-chunk [128,1] columns
                # (rank-1 matmuls): used as the per-partition ACT drain scale
                pb = ps2.tile([128, CO], F32, tag="uh")
                for c in range(4):
                    nc.tensor.matmul(pb[:, c:c + 1],
                                     demod_row[0:1, 128 * c:128 * c + 128],
                                     ones_row[0:1, 0:1], start=True, stop=True)
                nc.vector.tensor_copy(out=dcol, in_=pb[:, 0:4])

            # ---- phase 1a: winograd transform + GEMM + stage-1 inverse.
            # xi-outer so T lives briefly; W-hat streamed from DRAM per
            # (xi,nu). M[co128, 961] accumulates over ci chunks in PSUM;
            # stage-1 (P_r,nu = sum_xi AT[r,xi] M_xi,nu) drains M
            # incrementally. Pool cannot read PSUM, so drains land on ACT
            # (copies) and DVE (psum-reads); Pool gets SBUF-only combines.
            nb = 0 if (variant or "").startswith("p2") else NB
            if nb:
                with tc.tile_pool(name="tpool", bufs=2) as tpool, \
                     tc.tile_pool(name="vpool", bufs=2) as vpool, \
                     tc.tile_pool(name="wpool", bufs=2) as wpool, \
                     tc.tile_pool(name="spool", bufs=2) as spool, \
                     tc.tile_pool(name="mps", bufs=3, space="PSUM") as mps:
                    for xi in range(4):
                        a0, a1, top = BT_COMBO[xi]
                        # T layout (w, ty) so pass-W reads are ty-packed
                        tt = tpool.tile([128, 4 * 31 * 64], BF16, tag="t")
                        for t in range(4):
                            # x is col-major (w, h): T_xi[ci,(w,ty)] =
                            # x[ci,w,2ty+a0] op x[ci,w,2ty+a1]; inner dim
                            # is the stride-2 h-downsample (fast on DVE)
                            xa = [bass.AP(tensor=xts[t].tensor,
                                          offset=xts[t].offset + a,
                                          ap=[xts[t].ap[0], [64, 64], [2, 31]])
                                  for a in (a0, a1)]
                            to = bass.AP(tensor=tt.tensor,
                                         offset=tt.offset + t * 31 * 64,
                                         ap=[tt.ap[0], [31, 64], [1, 31]])
                            nc.vector.tensor_tensor(out=to, in0=xa[0], in1=xa[1], op=top)
                        wt_t = wpool.tile([128, 16 * CO], BF16, tag="w")
                        nc.sync.dma_start(out=wt_t, in_=wt_d[xi])
                        for nu in range(4):
                            b0, b1, bop = BT_COMBO[nu]
                            vt = vpool.tile([128, 4 * NT2], BF16, tag="v")
                            for t in range(4):
                                # V[ci,(tx,ty)] = T[ci,2tx+b0,ty] op T[ci,2tx+b1,ty]
                                ta = [bass.AP(tensor=tt.tensor,
                                              offset=tt.offset + t * 31 * 64 + b * 31,
                                              ap=[tt.ap[0], [62, 31], [1, 31]])
                                      for b in (b0, b1)]
                                vo = bass.AP(tensor=vt.tensor,
                                             offset=vt.offset + t * NT2,
                                             ap=[vt.ap[0], [31, 31], [1, 31]])
                                nc.vector.tensor_tensor(out=vo, in0=ta[0], in1=ta[1], op=bop)
                            for c in range(4):
                                mt = mps.tile([128, 1024], F32, tag="m")
                                for t in range(4):
                                    wof = nu * 4 * CO + t * CO + c * 128
                                    lhs = wt_t[:, wof:wof + 128]
                                    for n0, n1 in ((0, 512), (512, NT2)):
                                        nc.tensor.matmul(
                                            mt[:, n0:n1],
                                            lhs, vt[:, t * NT2 + n0:t * NT2 + n1],
                                            start=(t == 0), stop=(t == 3))
                                # stage-1: AT = [[1,1,1,0],[0,1,-1,-1]].
                                # M drains on ACT only (DVE psum reads are
                                # ~3x slower on HW), demod applied as the
                                # per-partition drain scale; P combines are
                                # packed bf16 on DVE.
                                p = pts[c]
                                p0 = p[:, 0 + nu * NT2:0 + nu * NT2 + NT2]
                                p1 = p[:, 4 * NT2 + nu * NT2:4 * NT2 + nu * NT2 + NT2]
                                m_ = mt[:, 0:NT2]
                                dsc = dcol[:, c:c + 1]
                                if xi == 0:
                                    nc.scalar.mul(out=p0, in_=m_, mul=dsc)
                                elif xi == 1:
                                    nc.scalar.mul(out=p1, in_=m_, mul=dsc)
                                    nc.vector.tensor_add(p0, p0, p1)
                                elif xi == 2:
                                    sc = spool.tile([128, NT2], BF16, tag="sc")
                                    nc.scalar.mul(out=sc, in_=m_, mul=dsc)
                                    nc.vector.tensor_add(p0, p0, sc)
                                    nc.vector.tensor_sub(p1, p1, sc)
                                else:
                                    sc = spool.tile([128, NT2], BF16, tag="sc")
                                    nc.scalar.mul(out=sc, in_=m_, mul=dsc)
                                    nc.vector.tensor_sub(p1, p1, sc)

            # ---- phase 1b: stage-2 inverse (Y_rs = sum_nu AT[s,nu] P_r,nu),
            # then per-tx-block: PE identity-transposes scatter Y into ct
            # tiles [(w-col s)*64 + h, co], demod+bias epilogue, H-up,
            # u1 DMA. ----
            cts = {}
            psb_ref = []

            def emit_ct(j):
                # transpose via matmul with identity rhs: out = lhsT.T @ I.
                # lhsT = Y[co128, (ty,r) strided slice at (s, tx=j)]:
                # M enumerates (ty outer, r inner) = h = 2ty+r.
                pool = psb_ref[0] if psb_ref else ps2
                ct_ps = pool.tile([128, CO], F32, tag="uh")
                for c in range(4):
                    y = yts[c]
                    for s in range(2):
                        lhs = bass.AP(
                            tensor=y.tensor,
                            offset=y.offset + s * 2 * NT2 + j * 62,
                            ap=[y.ap[0], [1, 62]])
                        nc.tensor.matmul(
                            ct_ps[64 * s:64 * s + 62, c * 128:c * 128 + 128],
                            lhs, eye_t, start=True, stop=True)
                ct = cpool.tile([128, CO], BF16, tag="c")
                nc.scalar.copy(out=ct, in_=ct_ps)
                cts[j] = ct

            ustage = {}

            def emit_hup(j):
                # H-up pair: lo (w=2j, ct rows 0:62) and hi (w=2j+1,
                # rows 64:126) run as concurrent row-tiles. Drained into a
                # 4-j staging tile; one 1 MiB DMA per batch (issues have
                # ~1.6us fixed cost on HW).
                ct = cts.pop(j)
                pool = psb_ref[0] if psb_ref else ps2
                pu0 = pool.tile([128, CO], F32, tag="uh")
                pu1 = pool.tile([128, CO], F32, tag="uh")
                nc.tensor.matmul(pu0, ulo_t[0:62, :], ct[0:62, :],
                                 start=True, stop=True)
                nc.tensor.matmul(pu1, uhi_t[64:126, :], ct[64:126, :],
                                 start=True, stop=True)
                jb = j // 4
                if j % 4 == 0:
                    ustage[jb] = upool.tile([128, 4 * 2 * CO], BF16, tag="u1",
                                            name=f"us{jb}")
                ut = ustage[jb]
                for dlt, pu in ((0, pu0), (1, pu1)):
                    sl = (2 * (j % 4) + dlt) * CO
                    nc.scalar.copy(out=ut[:, sl:sl + CO], in_=pu)
                jlast = NB - 1 if nb == NB else nb - 1
                if j % 4 == 3 or j == jlast:
                    n = 2 * (j % 4) + 2
                    utv = ut.rearrange("p (n c) -> p n c", c=CO)
                    with nc.allow_non_contiguous_dma(reason="u1 batch"):
                        nc.sync.dma_start(
                            out=u1_d[8 * jb:8 * jb + n, :, :].transpose([1, 0, 2]),
                            in_=utv[:, 0:n])
                    ustage.pop(jb)

            if nb:
                xctx.close()  # free x tiles before u2 prefetch allocates
                ypool = yctx.enter_context(tc.tile_pool(name="ypool", bufs=1))
                psb_ref.append(yctx.enter_context(
                    tc.tile_pool(name="psb", bufs=6, space="PSUM")))
                yts.extend(ypool.tile([128, 4 * NT2], BF16, tag=f"y{c}",
                                      name=f"y{c}") for c in range(4))
                # stage-2: per (co-chunk, r, s): 2 chained bf16 adds.
                # Y layout (s, tx, h) with h = 2*ty + r interleaved, so the
                # phase-1b transpose lhsT is a contiguous 62-wide slice.
                for c in range(4):
                    p, y = pts[c], yts[c]
                    for r in range(2):
                        pr = [p[:, (4 * r + nu) * NT2:(4 * r + nu + 1) * NT2]
                              for nu in range(4)]
                        ys = [bass.AP(tensor=y.tensor,
                                      offset=y.offset + s * 2 * NT2 + r,
                                      ap=[y.ap[0], [62, 31], [2, 31]])
                              for s in range(2)]
                        bsc = bcol[:, c:c + 1]
                        eng = nc.vector
                        eng.tensor_add(ys[0], pr[0], pr[1])
                        eng.scalar_tensor_tensor(
                            out=ys[0], in0=ys[0], scalar=bsc, in1=pr[2],
                            op0=ALU.add, op1=ALU.add)
                        eng.tensor_sub(ys[1], pr[1], pr[2])
                        eng.scalar_tensor_tensor(
                            out=ys[1], in0=ys[1], scalar=bsc, in1=pr[3],
                            op0=ALU.add, op1=ALU.subtract)
                for j in range(nb):
                    emit_ct(j)
                    if j > 1:
                        emit_hup(j - 2)
                if nb > 1:
                    emit_hup(nb - 2)
                emit_hup(nb - 1)

        epool = ctx.enter_context(tc.tile_pool(name="epool", bufs=1))
        e_ts = []
        # ---- phase 2: W-up -> act -> W-down -> corner turn -> H-down, both
        # co halves together so every DMA moves 1 KiB-contiguous chunks.
        # Down matmuls (M=64) run as concurrent col-tile pairs (co half m on
        # psum bank m, partition halves = hu halves); all PSUM drains are
        # full-width [128, 512]. The hu corner turn bounces through DRAM per
        # 16-hu slab, into d1lo (g 0-3) / d1hi (g 4-7) so H-down's e-loads
        # for the hu-lo half overlap the hu-hi half's production.
        with tc.tile_pool(name="u2p", bufs=3) as u2p, \
             tc.tile_pool(name="apool", bufs=6) as apool, \
             tc.tile_pool(name="d1p", bufs=3) as d1p, \
             tc.tile_pool(name="psw", bufs=5, space="PSUM") as psw, \
             tc.tile_pool(name="psd", bufs=3, space="PSUM") as psd:
            for g in range(8 if variant != "p1" else 0):
                u2 = u2p.tile([128, 16 * CO], BF16, tag="u2")
                u2v = u2.rearrange("p (a c) -> p a c", a=16)
                nc.sync.dma_start(out=u2v[0:62],
                                  in_=u1_d[:, 16 * g:16 * g + 16, :])
                # slabF[p=(wd, hu-half) | f=(hu_in_half 8, co 512)]
                slabF = d1p.tile([128, 8 * CO], BF16, tag="slab")
                acts = {}

                def emit_wup(a):
                    # single-hu full-co slices in 1-bank psum units with
                    # per-slice prelus: shorter chain links, deeper psum
                    # pipelining (6 units in flight)
                    a_L = apool.tile([128, 1024], BF16, tag="A")
                    a_H = apool.tile([128, 1024], BF16, tag="A")
                    for s in range(2):
                        hl = (2 * a + s) * 512
                        pL = psw.tile([128, 512], F32, tag="pw")
                        nc.tensor.matmul(pL, ulo_t[0:62, :],
                                         u2[0:62, hl:hl + 512],
                                         start=True, stop=True)
                        nc.scalar.activation(out=a_L[:, 512 * s:512 * s + 512],
                                             in_=pL, func=AF.Prelu,
                                             scale=SQRT2, alpha=0.2)
                        pH = psw.tile([128, 512], F32, tag="pw")
                        nc.tensor.matmul(pH, ulo_t[0:62, :],
                                         u2[0:62, hl + 8 * 512:hl + 9 * 512],
                                         start=True, stop=True)
                        nc.scalar.activation(out=a_H[:, 512 * s:512 * s + 512],
                                             in_=pH, func=AF.Prelu,
                                             scale=SQRT2, alpha=0.2)
                    acts[a] = (a_L, a_H)

                def emit_wdown(a):
                    a_L, a_H = acts.pop(a)
                    for s in range(2):
                        pdw = psd.tile([128, 512], F32, tag="pdw")
                        nc.tensor.matmul(pdw[0:64, :],
                                         dmat_t, a_L[:, 512 * s:512 * s + 512],
                                         start=True, stop=True,
                                         tile_position=(0, 0))
                        nc.tensor.matmul(pdw[64:128, :],
                                         dmat_t, a_H[:, 512 * s:512 * s + 512],
                                         start=True, stop=True,
                                         tile_position=(0, 64))
                        if variant != "p2gs":
                            cpfn = (nc.vector.tensor_copy if (a == 3 and s == 1)
                                    else nc.scalar.copy)
                            cpfn(out=slabF[:, (2 * a + s) * 512:(2 * a + s + 1) * 512],
                                 in_=pdw)

                # W-down trails W-up by one block so the PE queue never
                # stalls on the Prelu drain
                for a in range(4):
                    emit_wup(a)
                    if a > 0:
                        emit_wdown(a - 1)
                emit_wdown(3)
                # corner turn for this 16-hu slab; overlaps next g.
                tgt = d1_d[g // 4]
                r0 = 16 * (g % 4)
                sv = slabF.rearrange("p (n c) -> p n c", c=CO)
                if variant not in ("p2gn", "p2gs"):
                    nc.sync.dma_start(out=tgt[:, r0:r0 + 8, :], in_=sv[0:64])
                    nc.sync.dma_start(out=tgt[:, r0 + 8:r0 + 16, :],
                                      in_=sv[64:128])
                if g == 3 and variant is None:
                    # d1lo complete: prefetch phase-3's hu-lo corner reads
                    # on the otherwise-idle Pool queue, under g 4-7
                    for cch in range(8):
                        e_t = epool.tile([128, 8 * CO], BF16, tag=f"e{cch}",
                                         name=f"e{cch}")
                        e_ts.append(e_t)
                        with nc.allow_non_contiguous_dma(reason="corner read"):
                            nc.gpsimd.dma_start(
                                out=e_t[0:64, :],
                                in_=d1_d[0][8 * cch:8 * cch + 8, :, :]
                                .transpose([1, 0, 2]))

        # H-down, streamed per 8-wd chunk; hu-lo/hi contributions in separate
        # psum tiles (row-tiled K=64 matmuls), summed by the DVE drain
        with tc.tile_pool(name="opool", bufs=4) as opool, \
             tc.tile_pool(name="psh", bufs=4, space="PSUM") as psh:
            if not e_ts:
                for cch in range(0 if (variant or "").startswith("p2g") or variant == "p1" else 8):
                    e_t = epool.tile([128, 8 * CO], BF16, tag=f"e{cch}",
                                     name=f"e{cch}")
                    e_ts.append(e_t)
                    with nc.allow_non_contiguous_dma(reason="corner turn read"):
                        nc.scalar.dma_start(
                            out=e_t[0:64, :],
                            in_=d1_d[0][8 * cch:8 * cch + 8, :, :]
                            .transpose([1, 0, 2]))
            for cch in range(len(e_ts)):
                e_t = e_ts[cch]
                with nc.allow_non_contiguous_dma(reason="corner turn read"):
                    nc.sync.dma_start(
                        out=e_t[64:128, :],
                        in_=d1_d[1][8 * cch:8 * cch + 8, :, :].transpose([1, 0, 2]))
                for p in range(2):
                    # 4 wd columns per wide psum tile: partitions (hd, wd
                    # pair-sel), banks = wd within pair; one full-width
                    # drain, 4 KiB-run output writes
                    pdh = psh.tile([128, 1024], F32, tag="pdh")
                    for s in range(4):
                        wl = (4 * p + s) * 512
                        nc.tensor.matmul(
                            pdh[64 * (s // 2):64 * (s // 2) + 64,
                                512 * (s % 2):512 * (s % 2) + 512],
                            dmat_t, e_t[:, wl:wl + 512],
                            start=True, stop=True,
                            tile_position=(0, 64 * (s // 2)))
                    ot = opool.tile([128, 1024], BF16, tag="o")
                    nc.scalar.copy(out=ot, in_=pdh)
                    wd0 = 8 * cch + 4 * p
                    otv = ot.rearrange("p (w c) -> p w c", c=CO)
                    eng = nc.scalar if p % 2 == 0 else nc.gpsimd
                    eng.dma_start(out=out_d[:, wd0:wd0 + 2, :],
                                  in_=otv[0:64])
                    eng.dma_start(out=out_d[:, wd0 + 2:wd0 + 4, :],
                                  in_=otv[64:128])

    nc.compile()
    return nc


_G_WINO = np.array([[1, 0, 0], [.5, .5, .5], [.5, -.5, .5], [0, 0, 1]],
                   np.float32)


def _host_prep(x, style, mod_w, mod_b, conv_w, act_b, up_filter, down_filter):
    x = np.ascontiguousarray(x, np.float32)
    style = np.asarray(style, np.float32)
    mod_w = np.asarray(mod_w, np.float32)
    mod_b = np.asarray(mod_b, np.float32)
    conv_w = np.asarray(conv_w, np.float32)
    act_b = np.asarray(act_b, np.float32)
    up_filter = np.asarray(up_filter, np.float64)
    down_filter = np.asarray(down_filter, np.float64)

    # FIR matrices (see upfirdn2d in the reference):
    #   up:   y[o] = sum_i fu[o + 3 - 2i] x[i],   fu = up_filter * 2
    #   down: y[o] = sum_u df[2o + 6 - u] x[u]
    fu = up_filter * UP
    U = np.zeros((HO, 2 * H), np.float32)
    for i in range(HO):
        for o in range(2 * H):
            t = o + 3 - 2 * i
            if 0 <= t < TAPS:
                U[i, o] = fu[t]
    D = np.zeros((2 * H, H), np.float32)
    for u in range(2 * H):
        for o in range(H):
            t = 2 * o + 6 - u
            if 0 <= t < TAPS:
                D[u, o] = down_filter[t]
    bf = ml_dtypes.bfloat16
    ulo = np.zeros((128, 128), np.float32)
    uhi = np.zeros((128, 128), np.float32)
    ulo[0:HO, :] = U
    uhi[64:64 + HO, :] = U
    ulo = ulo.astype(bf)
    uhi = uhi.astype(bf)
    dmat = np.ascontiguousarray(D).astype(bf)

    # prologue blob [128, 4108] f32:
    #   modw[0:2048] qmat[2048:4096] stylec[4096:4100] modb[4100:4104]
    #   actb-cols[4104:4108]
    modw_host = (mod_w * LIN_SCALE).T.reshape(4, 128, 512)
    modw_flat = modw_host.transpose(1, 0, 2).reshape(128, 2048)
    qmat_host = (conv_w.astype(np.float64) ** 2).sum(axis=(2, 3)).T \
        .reshape(4, 128, CO).astype(np.float32)
    qmat_flat = qmat_host.transpose(1, 0, 2).reshape(128, 2048)
    modb_host = (WSCALE * mod_b).reshape(4, 128).T
    actb_cols = act_b.reshape(4, 128).T
    preb = np.zeros((B, 128, 4108), np.float32)
    preb[:, :, 0:2048] = modw_flat
    preb[:, :, 2048:4096] = qmat_flat
    preb[:, :, 4096:4100] = style.reshape(B, 4, 128).transpose(0, 2, 1)
    preb[:, :, 4100:4104] = modb_host
    preb[:, :, 4104:4108] = actb_cols

    # consts blob [128, 448] bf16: ulo uhi dmat eye
    eye_host = np.eye(128, dtype=np.float32)
    cstb = np.ascontiguousarray(
        np.concatenate([ulo.astype(np.float32), uhi.astype(np.float32),
                        dmat.astype(np.float32), eye_host], axis=1)).astype(bf)

    # winograd weights: W[xi,nu][ci,co] = sum_ab G[xi,a] G[nu,b] w[co,ci,a,b]
    # laid out [xi, 128 ci_in_chunk, (nu, ci_chunk, co)]
    wt = np.einsum("xa,oiab,nb->xnoi", _G_WINO, conv_w, _G_WINO)
    wt = wt.reshape(4, 4, CO, 4, 128).transpose(0, 1, 4, 3, 2)  # x,n,ci_in,chunk,co
    wt_host = np.ascontiguousarray(
        wt.reshape(4, 4, 128, 4 * CO).transpose(0, 2, 1, 3)
        .reshape(4, 128, 16 * CO)).astype(bf)

    # col-major x: [128 ci_in, (chunk, w, h)]
    xp = x.reshape(B, 4, 128, 64, 64).transpose(0, 2, 1, 4, 3) \
        .reshape(B, 128, 4 * XPAD).astype(bf)

    shared = {"cstb": cstb, "wt": wt_host}
    in_maps = []
    for b in range(B):
        im = dict(shared)
        im["xt"] = np.ascontiguousarray(xp[b])
        im["preb"] = np.ascontiguousarray(preb[b])
        in_maps.append(im)
    return in_maps


def kernel(**inputs):
    _install_neff_cache()
    if "nc" not in _CACHE:
        _CACHE["nc"] = _build_nc()
    nc = _CACHE["nc"]
    in_maps = _host_prep(**inputs)
    trace = os.environ.get("AFC_TRACE", "0") == "1"
    res = run_bass_kernel_spmd(nc, in_maps, list(range(B)), trace=trace)
    _CACHE["last_result"] = res
    out = np.stack([r["o"].transpose(2, 0, 1) for r in res.results])
    return np.ascontiguousarray(out, np.float32)


def _install_neff_cache():
    """Disk-cache walrus compiles by BIR hash (compile is ~10 min)."""
    import hashlib
    import shutil as _sh
    from concourse import bass_utils as _bu
    from concourse import bass2jax as _bj
    if getattr(_bu, "_afc_cache_installed", False):
        return
    orig = _bu.compile_bir_kernel
    cache_dir = "/tmp/afc_neff_cache"
    os.makedirs(cache_dir, exist_ok=True)

    def cached(bir_json, tmpdir, neff_name="file.neff"):
        data = bir_json if isinstance(bir_json, bytes) else bir_json.encode()
        h = hashlib.sha256(data).hexdigest()[:24]
        cpath = os.path.join(cache_dir, h + ".neff")
        dst = os.path.join(tmpdir, neff_name)
        if os.path.exists(cpath):
            _sh.copy(cpath, dst)
            return dst
        p = orig(bir_json, tmpdir, neff_name)
        try:
            _sh.copy(p, cpath)
        except OSError:
            pass
        return p

    _bu.compile_bir_kernel = cached
    _bj.compile_bir_kernel = cached
    _bu._afc_cache_installed = True


def _make_runner(nc, in_maps, k=1):
    """Build a reusable jitted shard_map callable over 8 cores with
    device-resident inputs (mirrors bass2jax.run_bass_via_pjrt). With k>1
    the NEFF executes k times per dispatch, chained through the donated
    output operands so XLA cannot CSE or parallelize the calls."""
    import jax
    from jax.experimental.shard_map import shard_map
    from jax.sharding import Mesh, NamedSharding, PartitionSpec
    from concourse import bass2jax

    bass2jax.install_neuronx_cc_hook()
    partition_name = nc.partition_id_tensor.name if nc.partition_id_tensor else None
    in_names, out_names, out_avals, zero_outs = [], [], [], []
    for alloc in nc.m.functions[0].allocations:
        if not isinstance(alloc, mybir.MemoryLocationSet):
            continue
        name = alloc.memorylocations[0].name
        if alloc.kind == "ExternalInput":
            if name != partition_name:
                in_names.append(name)
        elif alloc.kind == "ExternalOutput":
            out_names.append(name)
            shape = tuple(alloc.tensor_shape)
            dtype = mybir.dt.np(alloc.dtype)
            out_avals.append(jax.core.ShapedArray(shape, dtype))
            zero_outs.append(np.zeros(shape, dtype))
    n_params = len(in_names)
    all_names = list(in_names) + out_names
    if partition_name is not None:
        all_names.append(partition_name)

    def _body(*args):
        ins = list(args[:n_params])
        outs = list(args[n_params:])
        for _ in range(k):
            operands = ins + outs
            if partition_name is not None:
                operands.append(bass2jax.partition_id_tensor())
            outs = list(bass2jax._bass_exec_p.bind(
                *operands, out_avals=tuple(out_avals),
                in_names=tuple(all_names), out_names=tuple(out_names),
                lowering_input_output_aliases=(), sim_require_finite=True,
                sim_require_nnan=True, nc=nc))
        return tuple(outs)

    n = len(in_maps)
    devices = jax.devices()[:n]
    mesh = Mesh(np.asarray(devices), ("core",))
    nin = n_params + len(out_names)
    f = jax.jit(shard_map(_body, mesh=mesh,
                          in_specs=(PartitionSpec("core"),) * nin,
                          out_specs=(PartitionSpec("core"),) * len(out_names),
                          check_rep=False), keep_unused=True)
    sh = NamedSharding(mesh, PartitionSpec("core"))
    args = [jax.device_put(
        np.concatenate([np.asarray(m[nm]) for m in in_maps], axis=0), sh)
        for nm in in_names]
    args += [jax.device_put(
        np.zeros((n * z.shape[0], *z.shape[1:]), z.dtype), sh)
        for z in zero_outs]
    return f, args


def _time_runner(f, args, iters):
    import time as _time
    for _ in range(2):
        jax.block_until_ready(f(*args))
    best = float("inf")
    for _ in range(iters):
        t0 = _time.perf_counter()
        jax.block_until_ready(f(*args))
        best = min(best, _time.perf_counter() - t0)
    return best


def time_kernel(iters=6, k1=8, k2=264, k=None, **inputs):
    # `k` accepted for backward compatibility with the original
    # 1x-vs-(1+k)x signature; the paired-difference method ignores it.
    """Per-execution time via in-kernel For_i repeat loops: the pipeline
    runs k1x and k2x per dispatch; the difference isolates device time from
    the ~80 ms axon dispatch overhead. Measurements are interleaved in
    (k1, k2) pairs and the median pair-difference is used, so slow drift
    (thermal/clock state) cancels. Returns ns."""
    global jax
    import jax
    import time as _time
    _install_neff_cache()
    in_maps = _host_prep(**inputs)
    f1, args = _make_runner(_build_nc(reps=k1), in_maps)
    f2, _ = _make_runner(_build_nc(reps=k2), in_maps)
    for f in (f1, f2):
        for _ in range(2):
            jax.block_until_ready(f(*args))
    diffs = []
    for _ in range(iters):
        t0 = _time.perf_counter()
        jax.block_until_ready(f1(*args))
        t1 = _time.perf_counter()
        jax.block_until_ready(f2(*args))
        t2 = _time.perf_counter()
        diffs.append(((t2 - t1) - (t1 - t0)) / (k2 - k1))
    diffs.sort()
    med = diffs[len(diffs) // 2]
    print("pair diffs (us):", " ".join(f"{d*1e6:.0f}" for d in diffs))
    return med * 1e9



# revision 54
# speedup vs baseline: 1.0402x; 1.0402x over previous
"""AliasFreeConv Trainium2 kernel.

Data-parallel over batch: 8 samples -> 8 NeuronCores, no collectives.
Per core:
  style modulation (PE matvec) scales x per-ci; demod (from a
  host-precomputed Q[ci,co] = sum_k conv_w^2) and act bias are applied
  per-co on the conv output epilogue ->
  per-sample 3x3 VALID conv via Winograd F(2x2,3x3): input transform
  B^T d B on DVE/Pool (two 1D passes), 16 per-(xi,nu) GEMMs with co on
  PSUM partitions (bf16, ~2.25x fewer PE cycles than direct), inverse
  transform stage-1 folded into the PSUM drains, stage-2 on DVE, then
  PE identity-matmul transposes scatter Y back into [2 w-cols x 64 h,
  co] conv-output tiles ->
  separable FIR up/down resampling as dense-matrix matmuls (U: 62->128,
  D: 128->64) with DRAM corner-turns between the up stages and between
  the down stages; Prelu(sqrt2*x, 0.2) on ACT.

All heavy matmuls in bf16 (fp32 PSUM accumulation); intermediates held in
bf16 to halve HBM traffic. The paired FIR matmuls use PE row/col tiling
(lo half on partitions 0-63, hi on 64-127) so each pair runs concurrently
on the 128x128 array.
"""
import math
import os
import numpy as np
from contextlib import ExitStack

import ml_dtypes

import concourse.bass as bass
import concourse.bacc as bacc
import concourse.tile as tile
from concourse import mybir
from concourse.bass_utils import run_bass_kernel_spmd

F32 = mybir.dt.float32
BF16 = mybir.dt.bfloat16
AF = mybir.ActivationFunctionType
ALU = mybir.AluOpType

B, CI, CO, H, W = 8, 512, 512, 64, 64
KS, TAPS, UP = 3, 12, 2
HO = H - 2                      # 62 valid conv outputs per axis
STYLE = 512
XPAD = 64 * 64                  # x tile free size: row-major (h, w)
NB = HO // 2                    # 31 w-column-pair blocks (= 31 tx tiles)
K9 = KS * KS
NT = 31                         # winograd tiles per axis
NT2 = NT * NT                   # 961 tiles
LIN_SCALE = 1.0 / math.sqrt(STYLE)
WSCALE = 1.0 / math.sqrt(CI * KS * KS)
EPS = 1e-8
SQRT2 = math.sqrt(2.0)

_CACHE: dict = {}
USE_GP_OPS = True    # use GPSIMD for elementwise transform ops
USE_GP_DMA = True    # use GPSIMD (SWDGE) for DMA issues


def _build_nc(reps=1, variant=None):
    # variant: None = full kernel; "p1" = phase 1 only; "noc" = single conv
    # matmul per block; "p2" = phase 2 only; "p2g*" = phase-2 g-loop only
    # (p2gd: prelu on DVE, p2gp: no prelu, p2gs: no slab copies, p2gn: no
    # corner DMA) -- dev-only timing bisection knobs
    nc = bacc.Bacc()

    # consolidated input blobs: DMA issues cost ~1.6us fixed each on HW,
    # so small tensors ride in two blobs (f32 prologue / bf16 consts)
    xt_d = nc.declare_dram_parameter("xt", [128, 4 * XPAD], BF16, isOutput=False)
    # preb: modw[0:2048] qmat[2048:4096] stylec[4096:4100] modb[4100:4104]
    #       actb-cols[4104:4108]
    preb_d = nc.declare_dram_parameter("preb", [128, 4108], F32, isOutput=False)
    # cstb: ulo[0:128] uhi[128:256] dmat[256:320] eye[320:448]
    cstb_d = nc.declare_dram_parameter("cstb", [128, 448], BF16, isOutput=False)
    wt_d = nc.declare_dram_parameter("wt", [4, 128, 4 * 4 * CO], BF16, isOutput=False)
    out_d = nc.declare_dram_parameter("o", [64, 64, CO], BF16, isOutput=True)

    u1_d = nc.dram_tensor("u1scratch", [HO, 128, CO], BF16)
    # corner-turned activations, split by hu half so the H-down lo-half
    # loads can start while the hi-half slabs are still being produced.
    d1_d = [nc.dram_tensor(f"d1scratch{h}", [64, 64, CO], BF16)  # [wd, hu, co]
            for h in range(2)]

    with ExitStack() as ctx:
        tc = ctx.enter_context(tile.TileContext(nc))
        if reps > 1:
            ctx.enter_context(tc.For_i(0, reps, 1))
        pp = ctx.enter_context(tc.tile_pool(name="persist", bufs=1))

        cst_t = pp.tile([128, 448], BF16)
        s_sc = pp.tile([128, 4], F32)
        dcol = pp.tile([128, 4], F32)   # demod, transposed per co-chunk
        bcol = pp.tile([128, 4], F32)   # act bias, transposed per co-chunk
        nc.sync.dma_start(out=cst_t, in_=cstb_d[:, :])
        ulo_t = cst_t[:, 0:128]
        uhi_t = cst_t[:, 128:256]
        dmat_t = cst_t[:, 256:320]
        eye_t = cst_t[:, 320:448]

        # Winograd F(2x2,3x3) combos. BT rows have exactly 2 nonzeros of
        # +-1, so each 1D transform plane is a single tensor_tensor op.
        #   BT = [[1,0,-1,0],[0,1,1,0],[0,-1,1,0],[0,1,0,-1]]
        BT_COMBO = [(0, 2, ALU.subtract), (1, 2, ALU.add),
                    (2, 1, ALU.subtract), (1, 3, ALU.subtract)]

        with tc.tile_pool(name="xw", bufs=1) as xw, \
             tc.tile_pool(name="cpool", bufs=3) as cpool, \
             tc.tile_pool(name="upool", bufs=3) as upool, \
             tc.tile_pool(name="ps2", bufs=2, space="PSUM") as ps2, \
             ExitStack() as xctx, ExitStack() as yctx:
            xpool = xctx.enter_context(tc.tile_pool(name="xpool", bufs=1))
            # ---- phase 1 resident tensors ----
            xall = xpool.tile([128, 4 * XPAD], BF16, tag="x", name="x")
            xts = [xall[:, t * XPAD:(t + 1) * XPAD] for t in range(4)]
            # P accumulators (stage-1 inverse transform), per co-chunk:
            # 8 planes (r, nu) of 961 tiles each
            pts = [xw.tile([128, 8 * NT2], BF16, tag=f"p{c}", name=f"p{c}") for c in range(4)]
            # Y planes (stage-2 output): allocated in phase 1b, after the
            # 1a transform pools free their space
            yts = []

            # ---- prologue: style modulation (scales x); demod per-co from
            # host-precomputed Q[ci,co] = sum_k conv_w^2 (f32 matvecs). ----
            with tc.tile_pool(name="pre", bufs=1) as pre:
                preb_t = pre.tile([128, 4108], F32)
                nc.sync.dma_start(out=preb_t, in_=preb_d[:, :])
                modw_t = [preb_t[:, 512 * t:512 * t + 512] for t in range(4)]
                styl_t = [preb_t[:, 4096 + t:4097 + t] for t in range(4)]
                modb_t = preb_t[:, 4100:4104]
                qm_t = [preb_t[:, 2048 + 512 * t:2048 + 512 * t + 512]
                        for t in range(4)]
                ones_row = pre.tile([1, 128], F32)
                sd_row = pre.tile([1, CO], F32)
                demod_row = pre.tile([1, CO], F32)
                s2_t = pre.tile([128, 4], F32)
                eps_t = pre.tile([1, 1], F32)
                nc.vector.memset(ones_row, 1.0)
                nc.vector.memset(eps_t, EPS * (1.0 + EPS) ** 2)
                nc.vector.tensor_copy(out=bcol, in_=preb_t[:, 4104:4108])
                nc.scalar.dma_start(out=xall, in_=xt_d[:, :])

                # s = style @ (mod_w*lin_scale).T; s_sc = wscale*(s + mod_b)
                pt_s = ps2.tile([128, CO], F32, tag="uh")
                for cib in range(4):
                    for dt_ in range(4):
                        nc.tensor.matmul(pt_s[:, cib:cib + 1],
                                         modw_t[dt_][:, cib * 128:(cib + 1) * 128],
                                         styl_t[dt_], start=(dt_ == 0), stop=(dt_ == 3))
                nc.vector.scalar_tensor_tensor(
                    out=s_sc, in0=pt_s[:, 0:4], scalar=WSCALE,
                    in1=modb_t, op0=ALU.mult, op1=ALU.add)

                # x <- x * s_sc (per-ci modulation on the input, not the weights)
                for t in range(4):
                    nc.vector.tensor_scalar_mul(xts[t], xts[t], s_sc[:, t:t + 1])

                # A[co] = sum_ci s_sc[ci]^2 * Q[ci, co]; demod = rsqrt(A+eps)
                nc.scalar.activation(out=s2_t, in_=s_sc, func=AF.Square)
                pa = ps2.tile([128, CO], F32, tag="uh")
                for t in range(4):
                    nc.tensor.matmul(pa[0:1, :], s2_t[:, t:t + 1], qm_t[t],
                                     start=(t == 0), stop=(t == 3))
                c2 = (1.0 + EPS) ** 2
                nc.scalar.activation(out=sd_row, in_=pa[0:1, :], func=AF.Sqrt,
                                     scale=c2, bias=eps_t)
                nc.vector.reciprocal(demod_row, sd_row)

                # transpose the demod row into per-co-chunk [128,1] columns
                # (rank-1 matmuls): used as the per-partition ACT drain scale
                pb = ps2.tile([128, CO], F32, tag="uh")
                for c in range(4):
                    nc.tensor.matmul(pb[:, c:c + 1],
                                     demod_row[0:1, 128 * c:128 * c + 128],
                                     ones_row[0:1, 0:1], start=True, stop=True)
                nc.vector.tensor_copy(out=dcol, in_=pb[:, 0:4])

            # ---- phase 1a: winograd transform + GEMM + stage-1 inverse.
            # xi-outer so T lives briefly; W-hat streamed from DRAM per
            # (xi,nu). M[co128, 961] accumulates over ci chunks in PSUM;
            # stage-1 (P_r,nu = sum_xi AT[r,xi] M_xi,nu) drains M
            # incrementally. Pool cannot read PSUM, so drains land on ACT
            # (copies) and DVE (psum-reads); Pool gets SBUF-only combines.
            nb = 0 if (variant or "").startswith("p2") else NB
            if nb:
                with tc.tile_pool(name="tpool", bufs=2) as tpool, \
                     tc.tile_pool(name="vpool", bufs=2) as vpool, \
                     tc.tile_pool(name="wpool", bufs=2) as wpool, \
                     tc.tile_pool(name="spool", bufs=2) as spool, \
                     tc.tile_pool(name="mps", bufs=3, space="PSUM") as mps:
                    for xi in range(4):
                        a0, a1, top = BT_COMBO[xi]
                        # T layout (w, ty) so pass-W reads are ty-packed
                        tt = tpool.tile([128, 4 * 31 * 64], BF16, tag="t")
                        for t in range(4):
                            # x is col-major (w, h): T_xi[ci,(w,ty)] =
                            # x[ci,w,2ty+a0] op x[ci,w,2ty+a1]; inner dim
                            # is the stride-2 h-downsample (fast on DVE)
                            xa = [bass.AP(tensor=xts[t].tensor,
                                          offset=xts[t].offset + a,
                                          ap=[xts[t].ap[0], [64, 64], [2, 31]])
                                  for a in (a0, a1)]
                            to = bass.AP(tensor=tt.tensor,
                                         offset=tt.offset + t * 31 * 64,
                                         ap=[tt.ap[0], [31, 64], [1, 31]])
                            nc.vector.tensor_tensor(out=to, in0=xa[0], in1=xa[1], op=top)
                        wt_t = wpool.tile([128, 16 * CO], BF16, tag="w")
                        nc.sync.dma_start(out=wt_t, in_=wt_d[xi])
                        for nu in range(4):
                            b0, b1, bop = BT_COMBO[nu]
                            vt = vpool.tile([128, 4 * NT2], BF16, tag="v")
                            for t in range(4):
                                # V[ci,(tx,ty)] = T[ci,2tx+b0,ty] op T[ci,2tx+b1,ty]
                                ta = [bass.AP(tensor=tt.tensor,
                                              offset=tt.offset + t * 31 * 64 + b * 31,
                                              ap=[tt.ap[0], [62, 31], [1, 31]])
                                      for b in (b0, b1)]
                                vo = bass.AP(tensor=vt.tensor,
                                             offset=vt.offset + t * NT2,
                                             ap=[vt.ap[0], [31, 31], [1, 31]])
                                nc.vector.tensor_tensor(out=vo, in0=ta[0], in1=ta[1], op=bop)
                            for c in range(4):
                                mt = mps.tile([128, 1024], F32, tag="m")
                                for t in range(4):
                                    wof = nu * 4 * CO + t * CO + c * 128
                                    lhs = wt_t[:, wof:wof + 128]
                                    for n0, n1 in ((0, 512), (512, NT2)):
                                        nc.tensor.matmul(
                                            mt[:, n0:n1],
                                            lhs, vt[:, t * NT2 + n0:t * NT2 + n1],
                                            start=(t == 0), stop=(t == 3))
                                # stage-1: AT = [[1,1,1,0],[0,1,-1,-1]].
                                # M drains on ACT only (DVE psum reads are
                                # ~3x slower on HW), demod applied as the
                                # per-partition drain scale; P combines are
                                # packed bf16 on DVE.
                                p = pts[c]
                                p0 = p[:, 0 + nu * NT2:0 + nu * NT2 + NT2]
                                p1 = p[:, 4 * NT2 + nu * NT2:4 * NT2 + nu * NT2 + NT2]
                                m_ = mt[:, 0:NT2]
                                dsc = dcol[:, c:c + 1]
                                if xi == 0:
                                    nc.scalar.mul(out=p0, in_=m_, mul=dsc)
                                elif xi == 1:
                                    nc.scalar.mul(out=p1, in_=m_, mul=dsc)
                                    nc.vector.tensor_add(p0, p0, p1)
                                elif xi == 2:
                                    sc = spool.tile([128, NT2], BF16, tag="sc")
                                    nc.scalar.mul(out=sc, in_=m_, mul=dsc)
                                    nc.vector.tensor_add(p0, p0, sc)
                                    nc.vector.tensor_sub(p1, p1, sc)
                                else:
                                    sc = spool.tile([128, NT2], BF16, tag="sc")
                                    nc.scalar.mul(out=sc, in_=m_, mul=dsc)
                                    nc.vector.tensor_sub(p1, p1, sc)

            # ---- phase 1b: stage-2 inverse (Y_rs = sum_nu AT[s,nu] P_r,nu),
            # then per-tx-block: PE identity-transposes scatter Y into ct
            # tiles [(w-col s)*64 + h, co], demod+bias epilogue, H-up,
            # u1 DMA. ----
            cts = {}
            psb_ref = []

            def emit_ct(j):
                # transpose via matmul with identity rhs: out = lhsT.T @ I.
                # lhsT = Y[co128, (ty,r) strided slice at (s, tx=j)]:
                # M enumerates (ty outer, r inner) = h = 2ty+r.
                pool = psb_ref[0] if psb_ref else ps2
                ct_ps = pool.tile([128, CO], F32, tag="uh")
                for c in range(4):
                    y = yts[c]
                    for s in range(2):
                        lhs = bass.AP(
                            tensor=y.tensor,
                            offset=y.offset + s * 2 * NT2 + j * 62,
                            ap=[y.ap[0], [1, 62]])
                        nc.tensor.matmul(
                            ct_ps[64 * s:64 * s + 62, c * 128:c * 128 + 128],
                            lhs, eye_t, start=True, stop=True)
                ct = cpool.tile([128, CO], BF16, tag="c")
                nc.scalar.copy(out=ct, in_=ct_ps)
                cts[j] = ct

            ustage = {}

            def emit_hup(j):
                # H-up pair: lo (w=2j, ct rows 0:62) and hi (w=2j+1,
                # rows 64:126) run as concurrent row-tiles. Drained into a
                # 4-j staging tile; one 1 MiB DMA per batch (issues have
                # ~1.6us fixed cost on HW).
                ct = cts.pop(j)
                pool = psb_ref[0] if psb_ref else ps2
                pu0 = pool.tile([128, CO], F32, tag="uh")
                pu1 = pool.tile([128, CO], F32, tag="uh")
                nc.tensor.matmul(pu0, ulo_t[0:62, :], ct[0:62, :],
                                 start=True, stop=True)
                nc.tensor.matmul(pu1, uhi_t[64:126, :], ct[64:126, :],
                                 start=True, stop=True)
                jb = j // 4
                if j % 4 == 0:
                    ustage[jb] = upool.tile([128, 4 * 2 * CO], BF16, tag="u1",
                                            name=f"us{jb}")
                ut = ustage[jb]
                for dlt, pu in ((0, pu0), (1, pu1)):
                    sl = (2 * (j % 4) + dlt) * CO
                    nc.scalar.copy(out=ut[:, sl:sl + CO], in_=pu)
                jlast = NB - 1 if nb == NB else nb - 1
                if j % 4 == 3 or j == jlast:
                    n = 2 * (j % 4) + 2
                    utv = ut.rearrange("p (n c) -> p n c", c=CO)
                    with nc.allow_non_contiguous_dma(reason="u1 batch"):
                        nc.sync.dma_start(
                            out=u1_d[8 * jb:8 * jb + n, :, :].transpose([1, 0, 2]),
                            in_=utv[:, 0:n])
                    ustage.pop(jb)

            if nb:
                xctx.close()  # free x tiles before u2 prefetch allocates
                ypool = yctx.enter_context(tc.tile_pool(name="ypool", bufs=1))
                psb_ref.append(yctx.enter_context(
                    tc.tile_pool(name="psb", bufs=6, space="PSUM")))
                yts.extend(ypool.tile([128, 4 * NT2], BF16, tag=f"y{c}",
                                      name=f"y{c}") for c in range(4))
                # stage-2: per (co-chunk, r, s): 2 chained bf16 adds.
                # Y layout (s, tx, h) with h = 2*ty + r interleaved, so the
                # phase-1b transpose lhsT is a contiguous 62-wide slice.
                for c in range(4):
                    p, y = pts[c], yts[c]
                    for r in range(2):
                        pr = [p[:, (4 * r + nu) * NT2:(4 * r + nu + 1) * NT2]
                              for nu in range(4)]
                        ys = [bass.AP(tensor=y.tensor,
                                      offset=y.offset + s * 2 * NT2 + r,
                                      ap=[y.ap[0], [62, 31], [2, 31]])
                              for s in range(2)]
                        bsc = bcol[:, c:c + 1]
                        eng = nc.vector
                        eng.tensor_add(ys[0], pr[0], pr[1])
                        eng.scalar_tensor_tensor(
                            out=ys[0], in0=ys[0], scalar=bsc, in1=pr[2],
                            op0=ALU.add, op1=ALU.add)
                        eng.tensor_sub(ys[1], pr[1], pr[2])
                        eng.scalar_tensor_tensor(
                            out=ys[1], in0=ys[1], scalar=bsc, in1=pr[3],
                            op0=ALU.add, op1=ALU.subtract)
                for j in range(nb):
                    emit_ct(j)
                    if j > 1:
                        emit_hup(j - 2)
                if nb > 1:
                    emit_hup(nb - 2)
                emit_hup(nb - 1)

        # ---- phase 2: W-up -> act -> W-down -> corner turn -> H-down, both
        # co halves together so every DMA moves 1 KiB-contiguous chunks.
        # Down matmuls (M=64) run as concurrent col-tile pairs (co half m on
        # psum bank m, partition halves = hu halves); all PSUM drains are
        # full-width [128, 512]. The hu corner turn bounces through DRAM per
        # 16-hu slab, into d1lo (g 0-3) / d1hi (g 4-7) so H-down's e-loads
        # for the hu-lo half overlap the hu-hi half's production.
        with tc.tile_pool(name="u2p", bufs=3) as u2p, \
             tc.tile_pool(name="apool", bufs=6) as apool, \
             tc.tile_pool(name="d1p", bufs=3) as d1p, \
             tc.tile_pool(name="psw", bufs=5, space="PSUM") as psw, \
             tc.tile_pool(name="psd", bufs=3, space="PSUM") as psd:
            for g in range(8 if variant != "p1" else 0):
                u2 = u2p.tile([128, 16 * CO], BF16, tag="u2")
                u2v = u2.rearrange("p (a c) -> p a c", a=16)
                nc.sync.dma_start(out=u2v[0:62],
                                  in_=u1_d[:, 16 * g:16 * g + 16, :])
                # slabF[p=(wd, hu-half) | f=(hu_in_half 8, co 512)]
                slabF = d1p.tile([128, 8 * CO], BF16, tag="slab")
                acts = {}

                def emit_wup(a):
                    # single-hu full-co slices in 1-bank psum units with
                    # per-slice prelus: shorter chain links, deeper psum
                    # pipelining (6 units in flight)
                    a_L = apool.tile([128, 1024], BF16, tag="A")
                    a_H = apool.tile([128, 1024], BF16, tag="A")
                    for s in range(2):
                        hl = (2 * a + s) * 512
                        pL = psw.tile([128, 512], F32, tag="pw")
                        nc.tensor.matmul(pL, ulo_t[0:62, :],
                                         u2[0:62, hl:hl + 512],
                                         start=True, stop=True)
                        nc.scalar.activation(out=a_L[:, 512 * s:512 * s + 512],
                                             in_=pL, func=AF.Prelu,
                                             scale=SQRT2, alpha=0.2)
                        pH = psw.tile([128, 512], F32, tag="pw")
                        nc.tensor.matmul(pH, ulo_t[0:62, :],
                                         u2[0:62, hl + 8 * 512:hl + 9 * 512],
                                         start=True, stop=True)
                        nc.scalar.activation(out=a_H[:, 512 * s:512 * s + 512],
                                             in_=pH, func=AF.Prelu,
                                             scale=SQRT2, alpha=0.2)
                    acts[a] = (a_L, a_H)

                def emit_wdown(a):
                    a_L, a_H = acts.pop(a)
                    for s in range(2):
                        pdw = psd.tile([128, 512], F32, tag="pdw")
                        nc.tensor.matmul(pdw[0:64, :],
                                         dmat_t, a_L[:, 512 * s:512 * s + 512],
                                         start=True, stop=True,
                                         tile_position=(0, 0))
                        nc.tensor.matmul(pdw[64:128, :],
                                         dmat_t, a_H[:, 512 * s:512 * s + 512],
                                         start=True, stop=True,
                                         tile_position=(0, 64))
                        if variant != "p2gs":
                            cpfn = (nc.vector.tensor_copy if (a == 3 and s == 1)
                                    else nc.scalar.copy)
                            cpfn(out=slabF[:, (2 * a + s) * 512:(2 * a + s + 1) * 512],
                                 in_=pdw)

                # W-down trails W-up by one block so the PE queue never
                # stalls on the Prelu drain
                for a in range(4):
                    emit_wup(a)
                    if a > 0:
                        emit_wdown(a - 1)
                emit_wdown(3)
                # corner turn for this 16-hu slab; overlaps next g. Issued
                # on the Pool queue (sync carries u2, ACT is drain-bound).
                tgt = d1_d[g // 4]
                r0 = 16 * (g % 4)
                sv = slabF.rearrange("p (n c) -> p n c", c=CO)
                if variant not in ("p2gn", "p2gs"):
                    nc.gpsimd.dma_start(out=tgt[:, r0:r0 + 8, :], in_=sv[0:64])
                    nc.gpsimd.dma_start(out=tgt[:, r0 + 8:r0 + 16, :],
                                        in_=sv[64:128])

        # H-down, streamed per 8-wd chunk; hu-lo/hi contributions in separate
        # psum tiles (row-tiled K=64 matmuls), summed by the DVE drain
        with tc.tile_pool(name="epool", bufs=1) as epool, \
             tc.tile_pool(name="opool", bufs=4) as opool, \
             tc.tile_pool(name="psh", bufs=4, space="PSUM") as psh:
            # hu-lo loads depend only on d1lo (g 0-3): prefetch them on the
            # scalar queue under phase 2's tail; hi loads gate on the last
            # corner turn and go on sync.
            e_ts = []
            for cch in range(0 if (variant or "").startswith("p2g") or variant == "p1" else 8):
                e_t = epool.tile([128, 8 * CO], BF16, tag=f"e{cch}",
                                 name=f"e{cch}")
                e_ts.append(e_t)
                with nc.allow_non_contiguous_dma(reason="corner turn read"):
                    nc.scalar.dma_start(
                        out=e_t[0:64, :],
                        in_=d1_d[0][8 * cch:8 * cch + 8, :, :].transpose([1, 0, 2]))
            for cch in range(len(e_ts)):
                e_t = e_ts[cch]
                with nc.allow_non_contiguous_dma(reason="corner turn read"):
                    nc.sync.dma_start(
                        out=e_t[64:128, :],
                        in_=d1_d[1][8 * cch:8 * cch + 8, :, :].transpose([1, 0, 2]))
                for p in range(2):
                    # 4 wd columns per wide psum tile: partitions (hd, wd
                    # pair-sel), banks = wd within pair; one full-width
                    # drain, 4 KiB-run output writes
                    pdh = psh.tile([128, 1024], F32, tag="pdh")
                    for s in range(4):
                        wl = (4 * p + s) * 512
                        nc.tensor.matmul(
                            pdh[64 * (s // 2):64 * (s // 2) + 64,
                                512 * (s % 2):512 * (s % 2) + 512],
                            dmat_t, e_t[:, wl:wl + 512],
                            start=True, stop=True,
                            tile_position=(0, 64 * (s // 2)))
                    ot = opool.tile([128, 1024], BF16, tag="o")
                    nc.scalar.copy(out=ot, in_=pdh)
                    wd0 = 8 * cch + 4 * p
                    otv = ot.rearrange("p (w c) -> p w c", c=CO)
                    eng = nc.scalar if p % 2 == 0 else nc.gpsimd
                    eng.dma_start(out=out_d[:, wd0:wd0 + 2, :],
                                  in_=otv[0:64])
                    eng.dma_start(out=out_d[:, wd0 + 2:wd0 + 4, :],
                                  in_=otv[64:128])

    nc.compile()
    return nc


_G_WINO = np.array([[1, 0, 0], [.5, .5, .5], [.5, -.5, .5], [0, 0, 1]],
                   np.float32)


def _host_prep(x, style, mod_w, mod_b, conv_w, act_b, up_filter, down_filter):
    x = np.ascontiguousarray(x, np.float32)
    style = np.asarray(style, np.float32)
    mod_w = np.asarray(mod_w, np.float32)
    mod_b = np.asarray(mod_b, np.float32)
    conv_w = np.asarray(conv_w, np.float32)
    act_b = np.asarray(act_b, np.float32)
    up_filter = np.asarray(up_filter, np.float64)
    down_filter = np.asarray(down_filter, np.float64)

    # FIR matrices (see upfirdn2d in the reference):
    #   up:   y[o] = sum_i fu[o + 3 - 2i] x[i],   fu = up_filter * 2
    #   down: y[o] = sum_u df[2o + 6 - u] x[u]
    fu = up_filter * UP
    U = np.zeros((HO, 2 * H), np.float32)
    for i in range(HO):
        for o in range(2 * H):
            t = o + 3 - 2 * i
            if 0 <= t < TAPS:
                U[i, o] = fu[t]
    D = np.zeros((2 * H, H), np.float32)
    for u in range(2 * H):
        for o in range(H):
            t = 2 * o + 6 - u
            if 0 <= t < TAPS:
                D[u, o] = down_filter[t]
    bf = ml_dtypes.bfloat16
    ulo = np.zeros((128, 128), np.float32)
    uhi = np.zeros((128, 128), np.float32)
    ulo[0:HO, :] = U
    uhi[64:64 + HO, :] = U
    ulo = ulo.astype(bf)
    uhi = uhi.astype(bf)
    dmat = np.ascontiguousarray(D).astype(bf)

    # prologue blob [128, 4108] f32:
    #   modw[0:2048] qmat[2048:4096] stylec[4096:4100] modb[4100:4104]
    #   actb-cols[4104:4108]
    modw_host = (mod_w * LIN_SCALE).T.reshape(4, 128, 512)
    modw_flat = modw_host.transpose(1, 0, 2).reshape(128, 2048)
    qmat_host = (conv_w.astype(np.float64) ** 2).sum(axis=(2, 3)).T \
        .reshape(4, 128, CO).astype(np.float32)
    qmat_flat = qmat_host.transpose(1, 0, 2).reshape(128, 2048)
    modb_host = (WSCALE * mod_b).reshape(4, 128).T
    actb_cols = act_b.reshape(4, 128).T
    preb = np.zeros((B, 128, 4108), np.float32)
    preb[:, :, 0:2048] = modw_flat
    preb[:, :, 2048:4096] = qmat_flat
    preb[:, :, 4096:4100] = style.reshape(B, 4, 128).transpose(0, 2, 1)
    preb[:, :, 4100:4104] = modb_host
    preb[:, :, 4104:4108] = actb_cols

    # consts blob [128, 448] bf16: ulo uhi dmat eye
    eye_host = np.eye(128, dtype=np.float32)
    cstb = np.ascontiguousarray(
        np.concatenate([ulo.astype(np.float32), uhi.astype(np.float32),
                        dmat.astype(np.float32), eye_host], axis=1)).astype(bf)

    # winograd weights: W[xi,nu][ci,co] = sum_ab G[xi,a] G[nu,b] w[co,ci,a,b]
    # laid out [xi, 128 ci_in_chunk, (nu, ci_chunk, co)]
    wt = np.einsum("xa,oiab,nb->xnoi", _G_WINO, conv_w, _G_WINO)
    wt = wt.reshape(4, 4, CO, 4, 128).transpose(0, 1, 4, 3, 2)  # x,n,ci_in,chunk,co
    wt_host = np.ascontiguousarray(
        wt.reshape(4, 4, 128, 4 * CO).transpose(0, 2, 1, 3)
        .reshape(4, 128, 16 * CO)).astype(bf)

    # col-major x: [128 ci_in, (chunk, w, h)]
    xp = x.reshape(B, 4, 128, 64, 64).transpose(0, 2, 1, 4, 3) \
        .reshape(B, 128, 4 * XPAD).astype(bf)

    shared = {"cstb": cstb, "wt": wt_host}
    in_maps = []
    for b in range(B):
        im = dict(shared)
        im["xt"] = np.ascontiguousarray(xp[b])
        im["preb"] = np.ascontiguousarray(preb[b])
        in_maps.append(im)
    return in_maps


def kernel(**inputs):
    _install_neff_cache()
    if "nc" not in _CACHE:
        _CACHE["nc"] = _build_nc()
    nc = _CACHE["nc"]
    in_maps = _host_prep(**inputs)
    trace = os.environ.get("AFC_TRACE", "0") == "1"
    res = run_bass_kernel_spmd(nc, in_maps, list(range(B)), trace=trace)
    _CACHE["last_result"] = res
    out = np.stack([r["o"].transpose(2, 0, 1) for r in res.results])
    return np.ascontiguousarray(out, np.float32)


def _install_neff_cache():
    """Disk-cache walrus compiles by BIR hash (compile is ~10 min)."""
    import hashlib
    import shutil as _sh
    from concourse import bass_utils as _bu
    from concourse import bass2jax as _bj
    if getattr(_bu, "_afc_cache_installed", False):
        return
    orig = _bu.compile_bir_kernel
    cache_dir = "/tmp/afc_neff_cache"
    os.makedirs(cache_dir, exist_ok=True)

    def cached(bir_json, tmpdir, neff_name="file.neff"):
        data = bir_json if isinstance(bir_json, bytes) else bir_json.encode()
        h = hashlib.sha256(data).hexdigest()[:24]
        cpath = os.path.join(cache_dir, h + ".neff")
        dst = os.path.join(tmpdir, neff_name)
        if os.path.exists(cpath):
            _sh.copy(cpath, dst)
            return dst
        p = orig(bir_json, tmpdir, neff_name)
        try:
            _sh.copy(p, cpath)
        except OSError:
            pass
        return p

    _bu.compile_bir_kernel = cached
    _bj.compile_bir_kernel = cached
    _bu._afc_cache_installed = True


def _make_runner(nc, in_maps, k=1):
    """Build a reusable jitted shard_map callable over 8 cores with
    device-resident inputs (mirrors bass2jax.run_bass_via_pjrt). With k>1
    the NEFF executes k times per dispatch, chained through the donated
    output operands so XLA cannot CSE or parallelize the calls."""
    import jax
    from jax.experimental.shard_map import shard_map
    from jax.sharding import Mesh, NamedSharding, PartitionSpec
    from concourse import bass2jax

    bass2jax.install_neuronx_cc_hook()
    partition_name = nc.partition_id_tensor.name if nc.partition_id_tensor else None
    in_names, out_names, out_avals, zero_outs = [], [], [], []
    for alloc in nc.m.functions[0].allocations:
        if not isinstance(alloc, mybir.MemoryLocationSet):
            continue
        name = alloc.memorylocations[0].name
        if alloc.kind == "ExternalInput":
            if name != partition_name:
                in_names.append(name)
        elif alloc.kind == "ExternalOutput":
            out_names.append(name)
            shape = tuple(alloc.tensor_shape)
            dtype = mybir.dt.np(alloc.dtype)
            out_avals.append(jax.core.ShapedArray(shape, dtype))
            zero_outs.append(np.zeros(shape, dtype))
    n_params = len(in_names)
    all_names = list(in_names) + out_names
    if partition_name is not None:
        all_names.append(partition_name)

    def _body(*args):
        ins = list(args[:n_params])
        outs = list(args[n_params:])
        for _ in range(k):
            operands = ins + outs
            if partition_name is not None:
                operands.append(bass2jax.partition_id_tensor())
            outs = list(bass2jax._bass_exec_p.bind(
                *operands, out_avals=tuple(out_avals),
                in_names=tuple(all_names), out_names=tuple(out_names),
                lowering_input_output_aliases=(), sim_require_finite=True,
                sim_require_nnan=True, nc=nc))
        return tuple(outs)

    n = len(in_maps)
    devices = jax.devices()[:n]
    mesh = Mesh(np.asarray(devices), ("core",))
    nin = n_params + len(out_names)
    f = jax.jit(shard_map(_body, mesh=mesh,
                          in_specs=(PartitionSpec("core"),) * nin,
                          out_specs=(PartitionSpec("core"),) * len(out_names),
                          check_rep=False), keep_unused=True)
    sh = NamedSharding(mesh, PartitionSpec("core"))
    args = [jax.device_put(
        np.concatenate([np.asarray(m[nm]) for m in in_maps], axis=0), sh)
        for nm in in_names]
    args += [jax.device_put(
        np.zeros((n * z.shape[0], *z.shape[1:]), z.dtype), sh)
        for z in zero_outs]
    return f, args


def _time_runner(f, args, iters):
    import time as _time
    for _ in range(2):
        jax.block_until_ready(f(*args))
    best = float("inf")
    for _ in range(iters):
        t0 = _time.perf_counter()
        jax.block_until_ready(f(*args))
        best = min(best, _time.perf_counter() - t0)
    return best


def time_kernel(iters=6, k1=8, k2=264, k=None, **inputs):
    # `k` accepted for backward compatibility with the original
    # 1x-vs-(1+k)x signature; the paired-difference method ignores it.
    """Per-execution time via in-kernel For_i repeat loops: the pipeline
    runs k1x and k2x per dispatch; the difference isolates device time from
    the ~80 ms axon dispatch overhead. Measurements are interleaved in
    (k1, k2) pairs and the median pair-difference is used, so slow drift
    (thermal/clock state) cancels. Returns ns."""
    global jax
    import jax
    import time as _time
    _install_neff_cache()
    in_maps = _host_prep(**inputs)
    f1, args = _make_runner(_build_nc(reps=k1), in_maps)
    f2, _ = _make_runner(_build_nc(reps=k2), in_maps)
    for f in (f1, f2):
        for _ in range(2):
            jax.block_until_ready(f(*args))
    diffs = []
    for _ in range(iters):
        t0 = _time.perf_counter()
        jax.block_until_ready(f1(*args))
        t1 = _time.perf_counter()
        jax.block_until_ready(f2(*args))
        t2 = _time.perf_counter()
        diffs.append(((t2 - t1) - (t1 - t0)) / (k2 - k1))
    diffs.sort()
    med = diffs[len(diffs) // 2]
    print("pair diffs (us):", " ".join(f"{d*1e6:.0f}" for d in diffs))
    return med * 1e9

